# revision 9
# baseline (speedup 1.0000x reference)
"""Batched Bjorck orthogonalization on 8 TRN2 NeuronCores.

w: [64, 1024, 1024] f32. 13 iterations of W <- 1.5 W - 0.5 W (W^T W).
Sharding: batch dim across 8 cores (8 matrices per core), fully independent.

Math: single-pass fp32r (e8m11) matmuls on the PE array at 1 cycle/row.
Empirically (numpy e8m11 simulation) this yields ~7e-4 max rel error vs the
fp32 reference over 13 iterations - far inside the 2e-2 gate - while using
1/3 of the PE cycles of the previous tf32x3 (Dekker hi/lo) scheme.

Per-core per-matrix iteration (all on-chip, state in SBUF):
  G: A' = 1.5 I - 0.5 (W^T W)   [upper-triangle only: 37 of 64 128-blocks
     via width-trimmed PSUM tiles (>=256 wide keeps fp32r at 1 cyc/row);
     the -0.5 scale is folded into the PSUM->SBUF drain, +1.5I via DVE]
  R: lower blocks of A' reconstructed via 27 PE transposes
  U: W = W A'                   [128 matmuls; U reads only WT/A', so the
     drain overwrites W in place]
  T: refresh W^T                [64 PE transposes]
Modeled per-core device time (InstructionCostModel): ~5.3 ms for 8 matrices.
"""

import numpy as np

_NC_CACHE = {}

P = 128  # partitions
NMAT = 1024  # matrix dim
C = 8  # row chunks (NMAT / P)
FB = 512  # psum free-block width
NB = 2  # free blocks per 1024 (NMAT / FB)
ITERS = 13
PG_BUFS = 3
PU_BUFS = 3
PT_BUFS = 2

# G-phase tile plan: (m, col_start, n_eff). Covers the upper triangle
# (incl. diagonal) of A' in 128-col blocks, trimming tile width to skip
# below-diagonal blocks while keeping every tile >= 256 wide (fp32r rate).
# m=7 uses a 256-wide tile, computing lower block (7,6) directly.
G_PLAN = [
    (0, 0, 512), (0, 512, 512),
    (1, 128, 384), (1, 512, 512),
    (2, 256, 256), (2, 512, 512),
    (3, 384, 384), (3, 768, 256),
    (4, 512, 512),
    (5, 640, 384),
    (6, 768, 256),
    (7, 768, 256),
]

# Lower 128-blocks (mb, nb) not covered by G_PLAN, reconstructed by
# transposing the mirror upper block; grouped in runs of <=4 consecutive
# nb sharing one PSUM tile + one drain.
G_RECON = [
    (1, [0]),
    (2, [0, 1]),
    (3, [0, 1, 2]),
    (4, [0, 1, 2, 3]),
    (5, [0, 1, 2, 3]), (5, [4]),
    (6, [0, 1, 2, 3]), (6, [4, 5]),
    (7, [0, 1, 2, 3]), (7, [4, 5]),  # (7,6) computed directly in G
]


def _build(B, iters=ITERS, static=False):
    import concourse.bacc as bacc
    import concourse.bass as bass
    import concourse.mybir as mybir
    from concourse.tile import TileContext

    F32 = mybir.dt.float32
    F32R = mybir.dt.float32r
    COPY = mybir.ActivationFunctionType.Copy

    nc = bacc.Bacc("TRN2", target_bir_lowering=False, debug=False)
    w = nc.dram_tensor("w", [B, NMAT, NMAT], F32, kind="ExternalInput")
    o = nc.dram_tensor("o", [B, NMAT, NMAT], F32, kind="ExternalOutput")
    ide = nc.dram_tensor("ide", [P, P], F32, kind="ExternalInput")  # I
    # [Z128 | -3I | Z512]: sliced per-tile so the G accumulation gains -3I on
    # the diagonal; the -0.5 drain scale turns it into the +1.5I of A'.
    cneg = nc.dram_tensor("cneg", [P, 6 * P], F32, kind="ExternalInput")

    with TileContext(nc) as tc:
        with (
            tc.tile_pool(name="state", bufs=1) as st,
            tc.tile_pool(name="const", bufs=1) as cn,
            tc.tile_pool(name="tmp", bufs=3) as tp,
            tc.tile_pool(name="pg", bufs=PG_BUFS, space="PSUM") as pg,
            tc.tile_pool(name="pu", bufs=PU_BUFS, space="PSUM") as pu,
            tc.tile_pool(name="pt", bufs=PT_BUFS, space="PSUM") as pt,
        ):
            ide32 = cn.tile([P, P], F32, tag="ide32")
            ide_r = cn.tile([P, P], F32R, tag="ide_r")
            cn32 = cn.tile([P, 6 * P], F32, tag="cn32")
            cn_r = cn.tile([P, 6 * P], F32R, tag="cn_r")
            nc.sync.dma_start(ide32[:], ide.ap())
            nc.scalar.activation(ide_r[:], ide32[:], COPY)
            nc.sync.dma_start(cn32[:], cneg.ap())
            nc.scalar.activation(cn_r[:], cn32[:], COPY)

            from contextlib import nullcontext

            loop_cm = nullcontext(0) if static else tc.For_i(0, B)
            with loop_cm as ib:
              for _sib in range(B if static else 1):
                if static:
                    ib = _sib
                W = st.tile([P, C * NMAT], F32R, tag="W")
                WT = st.tile([P, C * NMAT], F32R, tag="WT")
                A = st.tile([P, C * NMAT], F32R, tag="A")

                # ---- load W (fp32) block-wise via tmp tiles -> f32r round
                for c in range(C):
                    for nb2 in range(NB):
                        s = slice(c * NMAT + nb2 * FB, c * NMAT + (nb2 + 1) * FB)
                        t32 = tp.tile([P, FB], F32, tag="t32")
                        nc.sync.dma_start(
                            t32[:],
                            w.ap()[
                                bass.ds(ib, 1),
                                c * P : (c + 1) * P,
                                nb2 * FB : (nb2 + 1) * FB,
                            ],
                        )
                        nc.scalar.activation(W[:, s], t32[:], COPY)

                def phase_T():
                    # WT[dst_c] = transpose of column-block dst_c of W
                    # drains alternate ACT/DVE to balance engines
                    for ti, (dc, half) in enumerate(
                        [(d, h) for d in range(C) for h in range(NB)]
                    ):
                        ptile = pt.tile([P, FB], F32R, tag="pt")
                        for q in range(4):
                            i = half * 4 + q  # source row-chunk
                            nc.tensor.transpose(
                                ptile[:, q * P : (q + 1) * P],
                                W[:, i * NMAT + dc * P : i * NMAT + (dc + 1) * P],
                                ide_r[:],
                            )
                        d = slice(
                            dc * NMAT + half * FB, dc * NMAT + (half + 1) * FB
                        )
                        if ti % 2 == 0:
                            nc.vector.tensor_copy(WT[:, d], ptile[:])
                        else:
                            nc.scalar.activation(WT[:, d], ptile[:], COPY)

                phase_T()

                for it in range(iters):
                    last = it == iters - 1
                    # ---- G: A' = 1.5 I - 0.5 W^T W (upper triangle + diag)
                    for m, cs, n_eff in G_PLAN:
                        g = pg.tile([P, FB], F32, tag="pg")
                        has_diag = cs <= m * P < cs + n_eff
                        for k in range(C):
                            sh = W[:, k * NMAT + m * P : k * NMAT + (m + 1) * P]
                            mh = W[:, k * NMAT + cs : k * NMAT + cs + n_eff]
                            nc.tensor.matmul(
                                g[:, :n_eff], sh, mh,
                                start=(k == 0),
                                stop=(k == C - 1 and not has_diag),
                            )
                        if has_diag:
                            # accumulate -3I at the diagonal position
                            doff = m * P - cs  # 0 or 128
                            nc.tensor.matmul(
                                g[:, :n_eff], ide_r[:],
                                cn_r[:, P - doff : P - doff + n_eff],
                                start=False, stop=True,
                            )
                        d = slice(m * NMAT + cs, m * NMAT + cs + n_eff)
                        nc.scalar.activation(A[:, d], g[:, :n_eff], COPY, scale=-0.5)
                    # ---- G recon: lower blocks = transpose of upper blocks
                    for ri, (mb, nbs) in enumerate(G_RECON):
                        n_r = len(nbs) * P
                        pr = pt.tile([P, FB], F32R, tag="pt")
                        for qi, nb in enumerate(nbs):
                            nc.tensor.transpose(
                                pr[:, qi * P : (qi + 1) * P],
                                A[:, nb * NMAT + mb * P : nb * NMAT + (mb + 1) * P],
                                ide_r[:],
                            )
                        dst = slice(mb * NMAT + nbs[0] * P, mb * NMAT + nbs[0] * P + n_r)
                        if ri % 2 == 0:
                            nc.vector.tensor_copy(A[:, dst], pr[:, :n_r])
                        else:
                            nc.scalar.activation(A[:, dst], pr[:, :n_r], COPY)
                    # ---- U: W = W A'
                    for nb2 in (1, 0):
                        for i in range(C):
                            u = pu.tile([P, FB], F32, tag="pu")
                            for j in range(C):
                                sh = WT[:, j * NMAT + i * P : j * NMAT + (i + 1) * P]
                                mh = A[:, j * NMAT + nb2 * FB : j * NMAT + (nb2 + 1) * FB]
                                nc.tensor.matmul(
                                    u[:], sh, mh,
                                    start=(j == 0), stop=(j == C - 1),
                                )
                            if last:
                                t32o = tp.tile([P, FB], F32, tag="t32")
                                nc.scalar.activation(t32o[:], u[:], COPY)
                                nc.sync.dma_start(
                                    o.ap()[
                                        bass.ds(ib, 1),
                                        i * P : (i + 1) * P,
                                        nb2 * FB : (nb2 + 1) * FB,
                                    ],
                                    t32o[:],
                                )
                            else:
                                d = slice(i * NMAT + nb2 * FB, i * NMAT + (nb2 + 1) * FB)
                                nc.scalar.activation(W[:, d], u[:], COPY)
                    if not last:
                        phase_T()
    nc.compile()
    return nc


def _get_nc(B, iters=ITERS):
    key = (B, iters)
    if key not in _NC_CACHE:
        _NC_CACHE[key] = _build(B, iters)
    return _NC_CACHE[key]


def kernel(w) -> np.ndarray:
    from concourse.bass_utils import run_bass_kernel_spmd

    w = np.ascontiguousarray(np.asarray(w, dtype=np.float32))
    assert w.shape == (64, NMAT, NMAT), w.shape
    B = 8  # matrices per core
    nc = _get_nc(B)
    ide = np.eye(P, dtype=np.float32)
    cneg = np.zeros((P, 6 * P), dtype=np.float32)
    cneg[:, P : 2 * P] = -3.0 * np.eye(P, dtype=np.float32)
    in_maps = [
        {"w": np.ascontiguousarray(w[c * B : (c + 1) * B]), "ide": ide, "cneg": cneg}
        for c in range(8)
    ]
    res = run_bass_kernel_spmd(nc, in_maps, core_ids=list(range(8)))
    return np.concatenate([res.results[c]["o"] for c in range(8)], axis=0)
